# revision 36
# baseline (speedup 1.0000x reference)
"""Bidirectional Mamba block — Trainium2 Bass/Tile kernel, 8-core data-parallel.

Sharding: batch B=8 -> one sample per NeuronCore, zero collectives.

v2 design (measured-rate driven):
- h stored t-major (bf16) via strided tensor_tensor_scan writes (strides are
  free on the scan's 2-cyc/elem cadence), so the C*h mult-cumsum custom op
  streams contiguously at 1 cyc/elem instead of 2.2.
- b4 = u*B and u = xc*dt run in bf16 2x_1p packed mode (0.53 ns/elem).
- P1 conv + silu moved to ACT (native Silu table); DVE does one add per kt.
- softplus = Exp then Ln on ACT (same table set as the scan's exp).
- xc / silu(z) / dt stay SBUF-resident in bf16 (no DRAM roundtrip).
- All DMA on hardware DGE queues (sync/scalar) — no gpsimd SWDGE (its
  descriptor generation contends with DVE 2-port ops for the shared port).
"""

import numpy as np

import concourse.bass as bass
import concourse.bacc as bacc
import concourse.mybir as mybir
from concourse import tile
from concourse import bass_utils

AL = mybir.AluOpType
AF = mybir.ActivationFunctionType
F32 = mybir.dt.float32
F32R = mybir.dt.float32r
BF16 = mybir.dt.bfloat16

NCORES = 8
MMF = 512
MMDT = F32R


class Dims:
    def __init__(self, L=1024, D=512, DI=2048, DS=16, DTR=32, DFF=2048, TC=512):
        self.L, self.D, self.DI, self.DS, self.DTR, self.DFF = L, D, DI, DS, DTR, DFF
        self.TC = TC
        self.NTC = L // TC
        self.KD = D // 128
        self.KI = DI // 128
        self.KF = DFF // 128
        self.RCH = 128 if TC % 128 == 0 else 64   # t-columns per mscan chunk
        assert TC % self.RCH == 0 and L % TC == 0 and DS == 16 and DTR == 32


DIMS = Dims()

# --------------------------------------------------------------- custom DVE op
_MSCAN = None


def _get_mscan():
    """out[k] = running_sum(in0[k] * in1[k]) along the flattened free stream."""
    global _MSCAN
    if _MSCAN is not None:
        return _MSCAN
    from concourse.dve_spec import Spec, Src0, Src1, scan as dve_scan, lower, AluOp
    from concourse.dve_ops import DveOp, OPS, CUSTOM_DVE_SPECS
    import concourse.dve_ops as dve_ops_mod
    from concourse.dve_uop import DveOpSpec

    name = "MAMBA_MULT_CUMSUM"
    for op in OPS:
        if op.name == name:
            _MSCAN = op
            return op

    def _ref(in0, in1, s0, s1, imm2):
        p = in0.shape[0]
        prod = np.asarray(in0, np.float32) * np.asarray(in1, np.float32)
        return np.cumsum(prod.reshape(p, -1), axis=1,
                         dtype=np.float32).reshape(in0.shape)

    spec = Spec(body=dve_scan(AluOp.ADD, Src0 * Src1), reference=_ref)
    row = max(dve_ops_mod._SUB_OPCODE_FOR_NAME.values()) + 1
    assert row < 0x20
    dve_ops_mod._SUB_OPCODE_FOR_NAME[name] = row
    shas = {}
    for ver in ("v3", "v4"):
        shas[ver] = DveOpSpec(name=name, opcode=row, uops=lower(spec, ver=ver),
                              rd1_en=True).sha(ver)
    final = DveOp(name, spec, subdim=False, uops_sha=shas)
    OPS.append(final)
    CUSTOM_DVE_SPECS[name] = spec
    _MSCAN = final
    return final


# -------------------------------------------------------------------- builder
def build_program(dm: Dims = DIMS):
    mscan = _get_mscan()
    nc = bacc.Bacc("TRN2", target_bir_lowering=False, debug=False)

    L, D, DI, DS, DTR = dm.L, dm.D, dm.DI, dm.DS, dm.DTR
    dram = {}

    def din(name, shape, dt=F32):
        dram[name] = nc.dram_tensor(name, list(shape), dt,
                                    kind="ExternalInput").ap()

    din("xT", (D, L), MMDT); din("xTr", (D, L), MMDT)
    din("ones", (128, 128), MMDT)
    din("sel", (DTR, 2 * DS * 128), MMDT)
    for p in ("m1_", "m2_"):
        din(p + "in_wT", (D, 2 * DI), MMDT)
        din(p + "xproj_wT", (DI, DTR + 2 * DS), MMDT)
        din(p + "dt_wT", (DTR, DI), MMDT)
        din(p + "out_wT", (DI, D), MMDT)
        din(p + "A", (DI, DS))                      # -exp(A_log)
        din(p + "dt_b", (128, dm.KI))
        din(p + "cw0", (128, dm.KI))
        din(p + "cw1", (128, dm.KI))
        din(p + "cb", (128, dm.KI))
        din(p + "Dp", (128, dm.KI))
    din("ln_g", (128, dm.KD)); din("ln_b", (128, dm.KD))
    din("c1_wT", (D, dm.DFF), MMDT); din("c1_b", (128, dm.KF))
    din("c2_wT", (dm.DFF, D), MMDT); din("c2_b", (128, dm.KD))
    outT = nc.dram_tensor("outT", [D, L], F32, kind="ExternalOutput").ap()

    with tile.TileContext(nc) as tc_:
        _emit(nc, tc_, dram, outT, dm, mscan)
    nc.compile()
    return nc


def _emit(nc, tc_, dram, outT, dm, mscan):
    from contextlib import ExitStack
    L, D, DI, DS, DTR, DFF, TC, NTC = (dm.L, dm.D, dm.DI, dm.DS, dm.DTR,
                                       dm.DFF, dm.TC, dm.NTC)
    KD, KI, KF, RCH = dm.KD, dm.KI, dm.KF, dm.RCH
    NRC = TC // RCH
    KHALF = max(1, min(8, KI // 2))
    mm = nc.tensor.matmul

    with ExitStack() as ctx:
        pers = ctx.enter_context(tc_.tile_pool(name="pers", bufs=1))
        wp = ctx.enter_context(tc_.tile_pool(name="wp", bufs=2))
        psmm = ctx.enter_context(tc_.tile_pool(name="psmm", bufs=2, space="PSUM"))
        psacc = ctx.enter_context(tc_.tile_pool(name="psacc", bufs=1, space="PSUM"))
        dpool = ctx.enter_context(tc_.tile_pool(name="dpool", bufs=1, space="DRAM"))

        ones_sb = pers.tile([128, 128], MMDT, tag="ones", name="ones")
        nc.sync.dma_start(ones_sb[:], dram["ones"][:])
        eps_sb = pers.tile([128, 1], F32, tag="eps", name="eps")
        nc.vector.memset(eps_sb[:], 1e-5)

        y_scr = [dpool.tile([D, L], F32, tag=f"y_scr{i}", name=f"y_scr{i}")
                 for i in range(2)]

        # ====================================================== SSM directions
        for di_ in range(2):
          with tc_.tile_pool(name=f"dirp{di_}", bufs=1) as dirp, \
               tc_.tile_pool(name=f"dsp{di_}", bufs=2) as sp:
            p = ("m1_", "m2_")[di_]
            xnm = ("xT", "xTr")[di_]
            A_sb = dirp.tile([128, KI * DS], F32, tag="A", name="A")
            nc.sync.dma_start(
                A_sb[:].rearrange("q (k s) -> q k s", k=KI),
                dram[p + "A"].rearrange("(k q) s -> q k s", q=128))
            vec = {}
            for nm in ("dt_b", "cw0", "cw1", "cb", "Dp"):
                vec[nm] = dirp.tile([128, KI], F32, tag=nm, name=nm)
                nc.sync.dma_start(vec[nm][:], dram[p + nm][:])
            xpw_sb = dirp.tile([128, KI * (DTR + 2 * DS)], MMDT, tag="xpw",
                               name="xpw")
            nc.sync.dma_start(
                xpw_sb[:].rearrange("q (k c) -> q k c", k=KI),
                dram[p + "xproj_wT"].rearrange("(k q) c -> q k c", q=128))

            sel_sb = dirp.tile([64, 2 * DS * 128], MMDT, tag="sel", name="sel")
            nc.sync.dma_start(sel_sb[32:64, :], dram["sel"][:])
            dtw_sb = dirp.tile([DTR, DI], MMDT, tag="dtw", name="dtw")
            nc.sync.dma_start(dtw_sb[:], dram[p + "dt_wT"][:])
            carry = dirp.tile([128, KI * DS], BF16, tag="carry", name="carry")
            carryB = dirp.tile([128, KI], F32, tag="carryB", name="carryB")
            nc.vector.memset(carryB[:], 0.0)
            bcB = dirp.tile([128, DS * TC], BF16, tag="bcB", name="bcB")
            bcC = dirp.tile([128, DS * TC], BF16, tag="bcC", name="bcC")
            # bf16 per-direction activations, SBUF-resident
            xc_sb = dirp.tile([128, KI * TC], BF16, tag="xc_sb", name="xc_sb")
            sz_sb = dirp.tile([128, KI * TC], BF16, tag="sz_sb", name="sz_sb")
            dt_h = [dirp.tile([128, KHALF * TC], BF16, tag=f"dt_h{i}",
                              name=f"dt_h{i}") for i in range(2)]
            dbc_sb = dirp.tile([64, TC], MMDT, tag="dbc", name="dbc")

            for tcix in range(NTC):
                t0 = tcix * TC
                xtc = [sp.tile([128, TC], MMDT, tag=f"xtc{k}", name=f"xtc{k}",
                               bufs=1) for k in range(KD)]
                for k in range(KD):
                    nc.sync.dma_start(xtc[k][:],
                                      dram[xnm][k * 128:(k + 1) * 128, t0:t0 + TC])

                # ---- P1: x-part conv + silu; z-part silu; dbc ---------------
                dbc_ps = psacc.tile([64, TC], F32, tag="acc_dbc", name="acc_dbc")
                for kt in range(KI):
                    ps = psmm.tile([128, TC], F32, tag="mm", name="mm")
                    w4 = wp.tile([128, KD * 128], MMDT, tag="w_in", name="w_in")
                    nc.sync.dma_start(
                        w4[:].rearrange("q (k e) -> q k e", k=KD),
                        dram[p + "in_wT"].rearrange("(k q) e -> q k e", q=128)
                        [:, :, kt * 128:(kt + 1) * 128])
                    for nk in range(0, TC, MMF):
                        nn = min(MMF, TC - nk)
                        for k in range(KD):
                            mm(ps[:, nk:nk + nn], w4[:, k * 128:(k + 1) * 128],
                               xtc[k][:, nk:nk + nn],
                               start=(k == 0), stop=(k == KD - 1))
                    # s1 = ps*cw1 + cb   (ACT, PSUM->SBUF)
                    s1 = sp.tile([128, TC], F32, tag="cv1", name="cv1", bufs=1)
                    nc.scalar.activation(s1[:], ps[:], AF.Identity,
                                         bias=vec["cb"][:, kt:kt + 1],
                                         scale=vec["cw1"][:, kt:kt + 1])
                    # p0 = ps*cw0        (ACT, PSUM->SBUF)
                    p0 = sp.tile([128, TC], F32, tag="cv0", name="cv0", bufs=1)
                    nc.scalar.activation(p0[:], ps[:], AF.Copy,
                                         scale=vec["cw0"][:, kt:kt + 1])
                    # v = shift(p0) + s1 (DVE)
                    v = sp.tile([128, TC], F32, tag="cv2", name="cv2", bufs=2)
                    nc.vector.tensor_tensor(v[:, 1:TC], p0[:, 0:TC - 1],
                                            s1[:, 1:TC], AL.add)
                    nc.vector.tensor_tensor(v[:, 0:1], carryB[:, kt:kt + 1],
                                            s1[:, 0:1], AL.add)
                    nc.vector.tensor_copy(carryB[:, kt:kt + 1], p0[:, TC - 1:TC])
                    # xc = silu(v): f32r for the dbc matmul; bf16 via DVE cast
                    xck = sp.tile([128, TC], MMDT, tag="xck", name="xck", bufs=2)
                    nc.scalar.activation(xck[:], v[:], AF.Silu)
                    nc.vector.tensor_copy(xc_sb[:, kt * TC:(kt + 1) * TC],
                                          xck[:].bitcast(F32))
                    for nk in range(0, TC, MMF):
                        nn = min(MMF, TC - nk)
                        mm(dbc_ps[:, nk:nk + nn],
                           xpw_sb[:, kt * 64:(kt + 1) * 64],
                           xck[:, nk:nk + nn],
                           start=(kt == 0), stop=(kt == KI - 1))
                nc.scalar.copy(dbc_sb[:], dbc_ps[:])

                # ---- B3: broadcast B (s-major bf16) / C (t-major bf16) ------
                for s in range(2 * DS):
                    bps = psmm.tile([128, TC], F32, tag="mm", name="mm")
                    for nk in range(0, TC, MMF):
                        nn = min(MMF, TC - nk)
                        mm(bps[:, nk:nk + nn],
                           sel_sb[32:64, s * 128:(s + 1) * 128],
                           dbc_sb[DTR:DTR + 2 * DS, nk:nk + nn],
                           start=True, stop=True)
                    if s < DS:
                        nc.scalar.activation(bcB[:, s * TC:(s + 1) * TC],
                                             bps[:], AF.Copy)
                    else:
                        # DVE is idle in this window; keep ACT free for the
                        # dt softplus chain that gates the scans.
                        si = s - DS
                        nc.vector.tensor_copy(
                            bcC[:].rearrange("q (t s) -> q s t", s=DS)[:, si, :],
                            bps[:])

                for kt in range(KI):           # z-part: silu only
                    zps = psmm.tile([128, TC], F32, tag="mm", name="mm")
                    w4 = wp.tile([128, KD * 128], MMDT, tag="w_in", name="w_in")
                    nc.sync.dma_start(
                        w4[:].rearrange("q (k e) -> q k e", k=KD),
                        dram[p + "in_wT"].rearrange("(k q) e -> q k e", q=128)
                        [:, :, DI + kt * 128:DI + (kt + 1) * 128])
                    for nk in range(0, TC, MMF):
                        nn = min(MMF, TC - nk)
                        for k in range(KD):
                            mm(zps[:, nk:nk + nn], w4[:, k * 128:(k + 1) * 128],
                               xtc[k][:, nk:nk + nn],
                               start=(k == 0), stop=(k == KD - 1))
                    nc.scalar.activation(sz_sb[:, kt * TC:(kt + 1) * TC],
                                         zps[:], AF.Silu)
                # ---- P2 per kt-half ----------------------------------------
                y_ps = [psacc.tile([128, TC], F32, tag=f"acc{k}", name=f"acc{k}")
                        for k in range(KD)]
                nhalves = (KI + KHALF - 1) // KHALF
                for kh in range(nhalves):
                    kts = range(kh * KHALF, min(KI, (kh + 1) * KHALF))
                    dth = dt_h[kh % 2]
                    for kt in kts:          # dt = softplus: Exp batch ...
                        ki = kt - kh * KHALF
                        dps = psmm.tile([128, TC], F32, tag="mm", name="mm")
                        for nk in range(0, TC, MMF):
                            nn = min(MMF, TC - nk)
                            mm(dps[:, nk:nk + nn],
                               dtw_sb[:, kt * 128:(kt + 1) * 128],
                               dbc_sb[0:DTR, nk:nk + nn],
                               start=True, stop=True)
                        nc.scalar.activation(dth[:, ki * TC:(ki + 1) * TC],
                                             dps[:], AF.Exp,
                                             bias=vec["dt_b"][:, kt:kt + 1])
                    for kt in kts:          # ... then Ln batch, in place
                        ki = kt - kh * KHALF
                        nc.scalar.activation(dth[:, ki * TC:(ki + 1) * TC],
                                             dth[:, ki * TC:(ki + 1) * TC],
                                             AF.Ln, bias=1.0)
                    for kt in kts:
                        ki = kt - kh * KHALF
                        dts = dth[:, ki * TC:(ki + 1) * TC]
                        xcs = xc_sb[:, kt * TC:(kt + 1) * TC]
                        # u = xc * dt  (bf16 2x)
                        u = sp.tile([128, TC], BF16, tag="u", name="u", bufs=1)
                        nc.vector.tensor_tensor(u[:], xcs, dts, AL.mult)
                        # b4 = u (bcast) * bcB  (bf16 2x, one op)
                        b4 = sp.tile([128, DS * TC], BF16, tag="b4", name="b4",
                                     bufs=1)
                        uv = u[:].rearrange("q (o t) -> q o t", o=1)
                        nc.vector.tensor_tensor(
                            b4[:].rearrange("q (s t) -> q s t", s=DS),
                            uv.to_broadcast((128, DS, TC)),
                            bcB[:].rearrange("q (s t) -> q s t", s=DS), AL.mult)
                        # 16 scans: a = exp(dt*A_s) on ACT; h t-major bf16
                        h = sp.tile([128, DS * TC], F32, tag="h", name="h",
                                    bufs=1)
                        for s in range(DS):
                            a = sp.tile([128, TC], F32, tag="a", name="a",
                                        bufs=2)
                            nc.scalar.activation(
                                a[:], dts, AF.Exp,
                                scale=A_sb[:, kt * DS + s:kt * DS + s + 1])
                            init = (0.0 if tcix == 0
                                    else carry[:, kt * DS + s:kt * DS + s + 1])
                            nc.vector.tensor_tensor_scan(
                                h[:, s:s + DS * (TC - 1) + 1:DS], a[:],
                                b4[:, s * TC:(s + 1) * TC], init,
                                AL.mult, AL.add)
                        nc.vector.tensor_copy(
                            carry[:, kt * DS:(kt + 1) * DS],
                            h[:, DS * (TC - 1):DS * TC])
                        # y = sum_s C*h via contiguous mult-cumsum + diffs
                        yv = sp.tile([128, TC], F32, tag="yv", name="yv", bufs=1)
                        R = sp.tile([128, RCH * DS], F32, tag="R", name="R",
                                    bufs=1)
                        for c in range(NRC):
                            tA = c * RCH
                            nc.vector._custom_dve(
                                mscan, out=R[:],
                                in0=h[:, tA * DS:(tA + RCH) * DS],
                                in1=bcC[:, tA * DS:(tA + RCH) * DS])
                            nc.vector.tensor_copy(yv[:, tA:tA + 1],
                                                  R[:, DS - 1:DS])
                            nc.vector.tensor_tensor(
                                yv[:, tA + 1:tA + RCH], R[:, 2 * DS - 1::DS],
                                R[:, DS - 1:(RCH - 1) * DS:DS], AL.subtract)
                        nc.vector.scalar_tensor_tensor(
                            yv[:], xcs, vec["Dp"][:, kt:kt + 1], yv[:],
                            AL.mult, AL.add)
                        g = sp.tile([128, TC], MMDT, tag="g", name="g", bufs=1)
                        nc.vector.tensor_tensor(
                            g[:], yv[:], sz_sb[:, kt * TC:(kt + 1) * TC],
                            AL.mult)
                        w4 = wp.tile([128, KD * 128], MMDT, tag="w_out",
                                     name="w_out")
                        nc.sync.dma_start(
                            w4[:], dram[p + "out_wT"][kt * 128:(kt + 1) * 128, :])
                        for k in range(KD):
                            for nk in range(0, TC, MMF):
                                nn = min(MMF, TC - nk)
                                mm(y_ps[k][:, nk:nk + nn],
                                   w4[:, k * 128:(k + 1) * 128],
                                   g[:, nk:nk + nn],
                                   start=(kt == 0), stop=(kt == KI - 1))
                for k in range(KD):
                    yo = sp.tile([128, TC], F32, tag="yo", name="yo", bufs=1)
                    nc.scalar.copy(yo[:], y_ps[k][:])
                    nc.sync.dma_start(
                        y_scr[di_][k * 128:(k + 1) * 128, t0:t0 + TC], yo[:])

        # ============================================================ phase C
        with tc_.tile_pool(name="cpool", bufs=1) as cp, \
             tc_.tile_pool(name="csp", bufs=2) as sp:
            ln_g = cp.tile([128, KD], F32, tag="ln_g", name="ln_g")
            ln_b = cp.tile([128, KD], F32, tag="ln_b", name="ln_b")
            c1b = cp.tile([128, KF], F32, tag="c1b", name="c1b")
            c2b = cp.tile([128, KD], F32, tag="c2b", name="c2b")
            for nm, t in (("ln_g", ln_g), ("ln_b", ln_b), ("c1_b", c1b),
                          ("c2_b", c2b)):
                nc.sync.dma_start(t[:], dram[nm][:])
            CH = min(MMF, L)

            def ln_chunk(in_tiles, out_tiles, nk):
                """LayerNorm over D for positions [nk, nk+CH), chunk-local."""
                sps = psacc.tile([1, CH], F32, tag="mmrow", name="mmrow")
                for k in range(KD):
                    mm(sps[:], ones_sb[:, 0:1], in_tiles[k][:, nk:nk + CH],
                       start=(k == 0), stop=(k == KD - 1))
                sums = cp.tile([1, CH], MMDT, tag="ln_srow", name="ln_srow")
                nc.scalar.copy(sums[:], sps[:])
                qps = psacc.tile([1, CH], F32, tag="mmrow", name="mmrow")
                for k in range(KD):
                    sq = sp.tile([128, CH], MMDT, tag="ln_sq", name="ln_sq")
                    nc.scalar.activation(sq[:], in_tiles[k][:, nk:nk + CH],
                                         AF.Square)
                    mm(qps[:], ones_sb[:, 0:1], sq[:],
                       start=(k == 0), stop=(k == KD - 1))
                sqs = cp.tile([1, CH], MMDT, tag="ln_qrow", name="ln_qrow")
                nc.scalar.copy(sqs[:], qps[:])
                mu = cp.tile([128, CH], F32, tag="ln_mu", name="ln_mu")
                inv = cp.tile([128, CH], F32, tag="ln_inv", name="ln_inv")
                mps = psmm.tile([128, CH], F32, tag="mm", name="mm")
                mm(mps[:], ones_sb[0:1, :], sums[:], start=True, stop=True)
                nc.vector.tensor_scalar(mu[:], mps[:], 1.0 / D, None, AL.mult)
                qrep = psmm.tile([128, CH], F32, tag="mm", name="mm")
                mm(qrep[:], ones_sb[0:1, :], sqs[:], start=True, stop=True)
                ex2 = sp.tile([128, CH], F32, tag="ln_ex2", name="ln_ex2")
                nc.vector.tensor_scalar(ex2[:], qrep[:], 1.0 / D, None, AL.mult)
                var = sp.tile([128, CH], F32, tag="ln_var", name="ln_var")
                nc.vector.tensor_tensor(var[:], mu[:], mu[:], AL.mult)
                nc.vector.tensor_tensor(var[:], ex2[:], var[:], AL.subtract)
                sd = sp.tile([128, CH], F32, tag="ln_sd", name="ln_sd")
                nc.scalar.activation(sd[:], var[:], AF.Sqrt, bias=eps_sb[:])
                nc.vector.reciprocal(inv[:], sd[:])
                for k in range(KD):
                    xm = sp.tile([128, CH], F32, tag="ln_xm", name="ln_xm")
                    nc.vector.tensor_tensor(xm[:], in_tiles[k][:, nk:nk + CH],
                                            mu[:], AL.subtract)
                    nc.vector.tensor_tensor(xm[:], xm[:], inv[:], AL.mult)
                    nc.vector.tensor_scalar(out_tiles[k][:, nk:nk + CH], xm[:],
                                            ln_g[:, k:k + 1], ln_b[:, k:k + 1],
                                            AL.mult, AL.add)

            y3p = [cp.tile([128, L], MMDT, tag=f"y3p{k}", name=f"y3p{k}")
                   for k in range(KD)]
            y3 = [cp.tile([128, L], MMDT, tag=f"y3_{k}", name=f"y3_{k}")
                  for k in range(KD)]
            outs = [cp.tile([128, L], MMDT, tag=f"o_{k}", name=f"o_{k}")
                    for k in range(KD)]
            ypre = y3p
            NFH = min(8, KF)
            for nk in range(0, L, CH):
                for k in range(KD):
                    xt = sp.tile([128, CH], MMDT, tag="c_x", name="c_x")
                    y1t = sp.tile([128, CH], F32, tag="c_y1", name="c_y1")
                    y2t = sp.tile([128, CH], F32, tag="c_y2", name="c_y2")
                    nc.sync.dma_start(
                        xt[:], dram["xT"][k * 128:(k + 1) * 128, nk:nk + CH])
                    nc.sync.dma_start(
                        y1t[:], y_scr[0][k * 128:(k + 1) * 128, nk:nk + CH])
                    nc.sync.dma_start(
                        y2t[:], y_scr[1][k * 128:(k + 1) * 128,
                                         L - nk - CH:L - nk])
                    nc.vector.tensor_tensor(y3p[k][:, nk:nk + CH], xt[:],
                                            y1t[:], AL.add)
                    nc.vector.tensor_tensor(y3p[k][:, nk:nk + CH],
                                            y3p[k][:, nk:nk + CH],
                                            y2t[:, ::-1], AL.add)
                ln_chunk(y3p, y3, nk)
                yacc = [psacc.tile([128, CH], F32, tag=f"acc{k}", name=f"acc{k}")
                        for k in range(KD)]
                for fh in range(KF // NFH):
                    hbuf = []
                    for f2 in range(NFH):
                        f = fh * NFH + f2
                        hps = psmm.tile([128, CH], F32, tag="mm", name="mm")
                        wc1 = wp.tile([128, KD * 128], MMDT, tag="w_in",
                                      name="w_c1")
                        nc.sync.dma_start(
                            wc1[:].rearrange("q (k e) -> q k e", k=KD),
                            dram["c1_wT"].rearrange("(k q) e -> q k e", q=128)
                            [:, :, f * 128:(f + 1) * 128])
                        for k in range(KD):
                            mm(hps[:], wc1[:, k * 128:(k + 1) * 128],
                               y3[k][:, nk:nk + CH],
                               start=(k == 0), stop=(k == KD - 1))
                        hb = sp.tile([128, CH], MMDT, tag=f"hb{f2}",
                                     name=f"hb{f2}", bufs=1)
                        nc.scalar.activation(hb[:], hps[:], AF.Relu,
                                             bias=c1b[:, f:f + 1])
                        hbuf.append(hb)
                    for f2 in range(NFH):
                        f = fh * NFH + f2
                        wc2 = wp.tile([128, KD * 128], MMDT, tag="w_out",
                                      name="w_c2")
                        nc.sync.dma_start(wc2[:],
                                          dram["c2_wT"][f * 128:(f + 1) * 128, :])
                        for k in range(KD):
                            mm(yacc[k][:], wc2[:, k * 128:(k + 1) * 128],
                               hbuf[f2][:],
                               start=(f == 0), stop=(f == KF - 1))
                for k in range(KD):
                    nc.vector.scalar_tensor_tensor(
                        ypre[k][:, nk:nk + CH], yacc[k][:], c2b[:, k:k + 1],
                        y3[k][:, nk:nk + CH], AL.add, AL.add)
                ln_chunk(ypre, outs, nk)
                for k in range(KD):
                    nc.sync.dma_start(
                        outT[k * 128:(k + 1) * 128, nk:nk + CH],
                        outs[k][:, nk:nk + CH].bitcast(F32))


# ------------------------------------------------------------------ host side
_PROG_CACHE = {}


def _get_prog():
    if "full" not in _PROG_CACHE:
        _PROG_CACHE["full"] = build_program(DIMS)
    return _PROG_CACHE["full"]


def host_prep(inputs, dm: Dims = DIMS):
    f = np.float32
    x = np.asarray(inputs["x"], dtype=f)
    KI, KD, KF = dm.KI, dm.KD, dm.KF

    def vt(v, n):
        return np.ascontiguousarray(np.asarray(v, f).reshape(n, 128).T)

    c = {}
    sel = np.zeros((dm.DTR, 2 * dm.DS * 128), f)
    for s in range(2 * dm.DS):
        sel[s, s * 128:(s + 1) * 128] = 1.0
    c["sel"] = sel
    c["ones"] = np.ones((128, 128), f)
    for p in ("m1_", "m2_"):
        c[p + "in_wT"] = np.ascontiguousarray(np.asarray(inputs[p + "in_w"], f).T)
        c[p + "xproj_wT"] = np.ascontiguousarray(
            np.asarray(inputs[p + "xproj_w"], f).T)
        c[p + "dt_wT"] = np.ascontiguousarray(np.asarray(inputs[p + "dt_w"], f).T)
        c[p + "out_wT"] = np.ascontiguousarray(
            np.asarray(inputs[p + "out_w"], f).T)
        c[p + "A"] = np.ascontiguousarray(-np.exp(np.asarray(inputs[p + "A_log"], f)))
        c[p + "dt_b"] = vt(inputs[p + "dt_b"], KI)
        cw = np.asarray(inputs[p + "conv_w"], f)
        c[p + "cw0"] = vt(cw[:, 0], KI)
        c[p + "cw1"] = vt(cw[:, 1], KI)
        c[p + "cb"] = vt(inputs[p + "conv_b"], KI)
        c[p + "Dp"] = vt(np.asarray(inputs[p + "Dp"], f), KI)
    c["ln_g"] = vt(inputs["ln_g"], KD)
    c["ln_b"] = vt(inputs["ln_b"], KD)
    c["c1_wT"] = np.ascontiguousarray(np.asarray(inputs["c1_w"], f).T)
    c["c1_b"] = vt(inputs["c1_b"], KF)
    c["c2_wT"] = np.ascontiguousarray(np.asarray(inputs["c2_w"], f).T)
    c["c2_b"] = vt(inputs["c2_b"], KD)

    in_maps = []
    for b in range(x.shape[0]):
        m = dict(c)
        m["xT"] = np.ascontiguousarray(x[b].T)
        m["xTr"] = np.ascontiguousarray(x[b][::-1].T)
        in_maps.append(m)
    return in_maps


def kernel(**inputs):
    nc = _get_prog()
    in_maps = host_prep(inputs)
    res = bass_utils.run_bass_kernel_spmd(nc, in_maps, core_ids=list(range(NCORES)))
    return np.stack([np.ascontiguousarray(o["outT"].T) for o in res.results], axis=0)


# revision 37
# speedup vs baseline: 1.0043x; 1.0043x over previous
"""Bidirectional Mamba block — Trainium2 Bass/Tile kernel, 8-core data-parallel.

Sharding: batch B=8 -> one sample per NeuronCore, zero collectives.

v2 design (measured-rate driven):
- h stored t-major (bf16) via strided tensor_tensor_scan writes (strides are
  free on the scan's 2-cyc/elem cadence), so the C*h mult-cumsum custom op
  streams contiguously at 1 cyc/elem instead of 2.2.
- b4 = u*B and u = xc*dt run in bf16 2x_1p packed mode (0.53 ns/elem).
- P1 conv + silu moved to ACT (native Silu table); DVE does one add per kt.
- softplus = Exp then Ln on ACT (same table set as the scan's exp).
- xc / silu(z) / dt stay SBUF-resident in bf16 (no DRAM roundtrip).
- All DMA on hardware DGE queues (sync/scalar) — no gpsimd SWDGE (its
  descriptor generation contends with DVE 2-port ops for the shared port).
"""

import numpy as np

import concourse.bass as bass
import concourse.bacc as bacc
import concourse.mybir as mybir
from concourse import tile
from concourse import bass_utils

AL = mybir.AluOpType
AF = mybir.ActivationFunctionType
F32 = mybir.dt.float32
F32R = mybir.dt.float32r
BF16 = mybir.dt.bfloat16

NCORES = 8
MMF = 512
MMDT = F32R


class Dims:
    def __init__(self, L=1024, D=512, DI=2048, DS=16, DTR=32, DFF=2048, TC=512):
        self.L, self.D, self.DI, self.DS, self.DTR, self.DFF = L, D, DI, DS, DTR, DFF
        self.TC = TC
        self.NTC = L // TC
        self.KD = D // 128
        self.KI = DI // 128
        self.KF = DFF // 128
        self.RCH = 128 if TC % 128 == 0 else 64   # t-columns per mscan chunk
        assert TC % self.RCH == 0 and L % TC == 0 and DS == 16 and DTR == 32


DIMS = Dims()

# --------------------------------------------------------------- custom DVE op
_MSCAN = None


def _get_mscan():
    """out[k] = running_sum(in0[k] * in1[k]) along the flattened free stream."""
    global _MSCAN
    if _MSCAN is not None:
        return _MSCAN
    from concourse.dve_spec import Spec, Src0, Src1, scan as dve_scan, lower, AluOp
    from concourse.dve_ops import DveOp, OPS, CUSTOM_DVE_SPECS
    import concourse.dve_ops as dve_ops_mod
    from concourse.dve_uop import DveOpSpec

    name = "MAMBA_MULT_CUMSUM"
    for op in OPS:
        if op.name == name:
            _MSCAN = op
            return op

    def _ref(in0, in1, s0, s1, imm2):
        p = in0.shape[0]
        prod = np.asarray(in0, np.float32) * np.asarray(in1, np.float32)
        return np.cumsum(prod.reshape(p, -1), axis=1,
                         dtype=np.float32).reshape(in0.shape)

    spec = Spec(body=dve_scan(AluOp.ADD, Src0 * Src1), reference=_ref)
    row = max(dve_ops_mod._SUB_OPCODE_FOR_NAME.values()) + 1
    assert row < 0x20
    dve_ops_mod._SUB_OPCODE_FOR_NAME[name] = row
    shas = {}
    for ver in ("v3", "v4"):
        shas[ver] = DveOpSpec(name=name, opcode=row, uops=lower(spec, ver=ver),
                              rd1_en=True).sha(ver)
    final = DveOp(name, spec, subdim=False, uops_sha=shas)
    OPS.append(final)
    CUSTOM_DVE_SPECS[name] = spec
    _MSCAN = final
    return final


# -------------------------------------------------------------------- builder
def build_program(dm: Dims = DIMS):
    mscan = _get_mscan()
    nc = bacc.Bacc("TRN2", target_bir_lowering=False, debug=False)

    L, D, DI, DS, DTR = dm.L, dm.D, dm.DI, dm.DS, dm.DTR
    dram = {}

    def din(name, shape, dt=F32):
        dram[name] = nc.dram_tensor(name, list(shape), dt,
                                    kind="ExternalInput").ap()

    din("xT", (D, L), MMDT); din("xTr", (D, L), MMDT)
    din("ones", (128, 128), MMDT)
    din("sel", (DTR, 2 * DS * 128), MMDT)
    for p in ("m1_", "m2_"):
        din(p + "in_wT", (D, 2 * DI), MMDT)
        din(p + "xproj_wT", (DI, DTR + 2 * DS), MMDT)
        din(p + "dt_wT", (DTR, DI), MMDT)
        din(p + "out_wT", (DI, D), MMDT)
        din(p + "A", (DI, DS))                      # -exp(A_log)
        din(p + "dt_b", (128, dm.KI))
        din(p + "cw0", (128, dm.KI))
        din(p + "cw1", (128, dm.KI))
        din(p + "cb", (128, dm.KI))
        din(p + "Dp", (128, dm.KI))
    din("ln_g", (128, dm.KD)); din("ln_b", (128, dm.KD))
    din("c1_wT", (D, dm.DFF), MMDT); din("c1_b", (128, dm.KF))
    din("c2_wT", (dm.DFF, D), MMDT); din("c2_b", (128, dm.KD))
    outT = nc.dram_tensor("outT", [D, L], F32, kind="ExternalOutput").ap()

    with tile.TileContext(nc) as tc_:
        _emit(nc, tc_, dram, outT, dm, mscan)
    nc.compile()
    return nc


def _emit(nc, tc_, dram, outT, dm, mscan):
    from contextlib import ExitStack
    L, D, DI, DS, DTR, DFF, TC, NTC = (dm.L, dm.D, dm.DI, dm.DS, dm.DTR,
                                       dm.DFF, dm.TC, dm.NTC)
    KD, KI, KF, RCH = dm.KD, dm.KI, dm.KF, dm.RCH
    NRC = TC // RCH
    KHALF = max(1, min(8, KI // 2))
    mm = nc.tensor.matmul

    with ExitStack() as ctx:
        pers = ctx.enter_context(tc_.tile_pool(name="pers", bufs=1))
        wp = ctx.enter_context(tc_.tile_pool(name="wp", bufs=2))
        psmm = ctx.enter_context(tc_.tile_pool(name="psmm", bufs=2, space="PSUM"))
        psacc = ctx.enter_context(tc_.tile_pool(name="psacc", bufs=1, space="PSUM"))
        dpool = ctx.enter_context(tc_.tile_pool(name="dpool", bufs=1, space="DRAM"))

        ones_sb = pers.tile([128, 128], MMDT, tag="ones", name="ones")
        nc.sync.dma_start(ones_sb[:], dram["ones"][:])
        eps_sb = pers.tile([128, 1], F32, tag="eps", name="eps")
        nc.vector.memset(eps_sb[:], 1e-5)

        y_scr = [dpool.tile([D, L], F32, tag=f"y_scr{i}", name=f"y_scr{i}")
                 for i in range(2)]

        # ====================================================== SSM directions
        for di_ in range(2):
          with tc_.tile_pool(name=f"dirp{di_}", bufs=1) as dirp, \
               tc_.tile_pool(name=f"dsp{di_}", bufs=2) as sp:
            p = ("m1_", "m2_")[di_]
            xnm = ("xT", "xTr")[di_]
            A_sb = dirp.tile([128, KI * DS], F32, tag="A", name="A")
            nc.sync.dma_start(
                A_sb[:].rearrange("q (k s) -> q k s", k=KI),
                dram[p + "A"].rearrange("(k q) s -> q k s", q=128))
            vec = {}
            for nm in ("dt_b", "cw0", "cw1", "cb", "Dp"):
                vec[nm] = dirp.tile([128, KI], F32, tag=nm, name=nm)
                nc.sync.dma_start(vec[nm][:], dram[p + nm][:])
            xpw_sb = dirp.tile([128, KI * (DTR + 2 * DS)], MMDT, tag="xpw",
                               name="xpw")
            nc.sync.dma_start(
                xpw_sb[:].rearrange("q (k c) -> q k c", k=KI),
                dram[p + "xproj_wT"].rearrange("(k q) c -> q k c", q=128))

            sel_sb = dirp.tile([64, 2 * DS * 128], MMDT, tag="sel", name="sel")
            nc.sync.dma_start(sel_sb[32:64, :], dram["sel"][:])
            dtw_sb = dirp.tile([DTR, DI], MMDT, tag="dtw", name="dtw")
            nc.sync.dma_start(dtw_sb[:], dram[p + "dt_wT"][:])
            carry = dirp.tile([128, KI * DS], BF16, tag="carry", name="carry")
            carryB = dirp.tile([128, KI], F32, tag="carryB", name="carryB")
            nc.vector.memset(carryB[:], 0.0)
            bcB = dirp.tile([128, DS * TC], BF16, tag="bcB", name="bcB")
            bcC = dirp.tile([128, DS * TC], BF16, tag="bcC", name="bcC")
            # bf16 per-direction activations, SBUF-resident
            xc_sb = dirp.tile([128, KI * TC], BF16, tag="xc_sb", name="xc_sb")
            sz_sb = dirp.tile([128, KI * TC], BF16, tag="sz_sb", name="sz_sb")
            dt_h = [dirp.tile([128, KHALF * TC], BF16, tag=f"dt_h{i}",
                              name=f"dt_h{i}") for i in range(2)]
            dbc_sb = dirp.tile([64, TC], MMDT, tag="dbc", name="dbc")

            for tcix in range(NTC):
                t0 = tcix * TC
                xtc = [sp.tile([128, TC], MMDT, tag=f"xtc{k}", name=f"xtc{k}",
                               bufs=1) for k in range(KD)]
                for k in range(KD):
                    nc.sync.dma_start(xtc[k][:],
                                      dram[xnm][k * 128:(k + 1) * 128, t0:t0 + TC])

                # ---- P1: x-part conv + silu; z-part silu; dbc ---------------
                dbc_ps = psacc.tile([64, TC], F32, tag="acc_dbc", name="acc_dbc")
                for kt in range(KI):
                    ps = psmm.tile([128, TC], F32, tag="mm", name="mm")
                    w4 = wp.tile([128, KD * 128], MMDT, tag="w_in", name="w_in")
                    nc.sync.dma_start(
                        w4[:].rearrange("q (k e) -> q k e", k=KD),
                        dram[p + "in_wT"].rearrange("(k q) e -> q k e", q=128)
                        [:, :, kt * 128:(kt + 1) * 128])
                    for nk in range(0, TC, MMF):
                        nn = min(MMF, TC - nk)
                        for k in range(KD):
                            mm(ps[:, nk:nk + nn], w4[:, k * 128:(k + 1) * 128],
                               xtc[k][:, nk:nk + nn],
                               start=(k == 0), stop=(k == KD - 1))
                    # s1 = ps*cw1 + cb   (ACT, PSUM->SBUF)
                    s1 = sp.tile([128, TC], F32, tag="cv1", name="cv1", bufs=1)
                    nc.scalar.activation(s1[:], ps[:], AF.Identity,
                                         bias=vec["cb"][:, kt:kt + 1],
                                         scale=vec["cw1"][:, kt:kt + 1])
                    # p0 = ps*cw0        (ACT, PSUM->SBUF)
                    p0 = sp.tile([128, TC], F32, tag="cv0", name="cv0", bufs=1)
                    nc.scalar.activation(p0[:], ps[:], AF.Copy,
                                         scale=vec["cw0"][:, kt:kt + 1])
                    # v = shift(p0) + s1 (DVE)
                    v = sp.tile([128, TC], F32, tag="cv2", name="cv2", bufs=2)
                    nc.vector.tensor_tensor(v[:, 1:TC], p0[:, 0:TC - 1],
                                            s1[:, 1:TC], AL.add)
                    nc.vector.tensor_tensor(v[:, 0:1], carryB[:, kt:kt + 1],
                                            s1[:, 0:1], AL.add)
                    nc.vector.tensor_copy(carryB[:, kt:kt + 1], p0[:, TC - 1:TC])
                    # xc = silu(v): f32r for the dbc matmul; bf16 via DVE cast
                    xck = sp.tile([128, TC], MMDT, tag="xck", name="xck", bufs=2)
                    nc.scalar.activation(xck[:], v[:], AF.Silu)
                    nc.vector.tensor_copy(xc_sb[:, kt * TC:(kt + 1) * TC],
                                          xck[:].bitcast(F32))
                    for nk in range(0, TC, MMF):
                        nn = min(MMF, TC - nk)
                        mm(dbc_ps[:, nk:nk + nn],
                           xpw_sb[:, kt * 64:(kt + 1) * 64],
                           xck[:, nk:nk + nn],
                           start=(kt == 0), stop=(kt == KI - 1))
                nc.scalar.copy(dbc_sb[:], dbc_ps[:])

                # ---- B3: broadcast B (s-major bf16) / C (t-major bf16) ------
                for s in range(2 * DS):
                    bps = psmm.tile([128, TC], F32, tag="mm", name="mm")
                    for nk in range(0, TC, MMF):
                        nn = min(MMF, TC - nk)
                        mm(bps[:, nk:nk + nn],
                           sel_sb[32:64, s * 128:(s + 1) * 128],
                           dbc_sb[DTR:DTR + 2 * DS, nk:nk + nn],
                           start=True, stop=True)
                    if s < DS:
                        nc.scalar.activation(bcB[:, s * TC:(s + 1) * TC],
                                             bps[:], AF.Copy)
                    else:
                        si = s - DS
                        nc.scalar.activation(
                            bcC[:].rearrange("q (t s) -> q s t", s=DS)[:, si, :],
                            bps[:], AF.Copy)

                for kt in range(KI):           # z-part: silu only
                    zps = psmm.tile([128, TC], F32, tag="mm", name="mm")
                    w4 = wp.tile([128, KD * 128], MMDT, tag="w_in", name="w_in")
                    nc.sync.dma_start(
                        w4[:].rearrange("q (k e) -> q k e", k=KD),
                        dram[p + "in_wT"].rearrange("(k q) e -> q k e", q=128)
                        [:, :, DI + kt * 128:DI + (kt + 1) * 128])
                    for nk in range(0, TC, MMF):
                        nn = min(MMF, TC - nk)
                        for k in range(KD):
                            mm(zps[:, nk:nk + nn], w4[:, k * 128:(k + 1) * 128],
                               xtc[k][:, nk:nk + nn],
                               start=(k == 0), stop=(k == KD - 1))
                    nc.scalar.activation(sz_sb[:, kt * TC:(kt + 1) * TC],
                                         zps[:], AF.Silu)
                # ---- P2 per kt-half ----------------------------------------
                y_ps = [psacc.tile([128, TC], F32, tag=f"acc{k}", name=f"acc{k}")
                        for k in range(KD)]
                nhalves = (KI + KHALF - 1) // KHALF
                for kh in range(nhalves):
                    kts = range(kh * KHALF, min(KI, (kh + 1) * KHALF))
                    dth = dt_h[kh % 2]
                    for kt in kts:          # dt = softplus: Exp batch ...
                        ki = kt - kh * KHALF
                        dps = psmm.tile([128, TC], F32, tag="mm", name="mm")
                        for nk in range(0, TC, MMF):
                            nn = min(MMF, TC - nk)
                            mm(dps[:, nk:nk + nn],
                               dtw_sb[:, kt * 128:(kt + 1) * 128],
                               dbc_sb[0:DTR, nk:nk + nn],
                               start=True, stop=True)
                        nc.scalar.activation(dth[:, ki * TC:(ki + 1) * TC],
                                             dps[:], AF.Exp,
                                             bias=vec["dt_b"][:, kt:kt + 1])
                    for kt in kts:          # ... then Ln batch, in place
                        ki = kt - kh * KHALF
                        nc.scalar.activation(dth[:, ki * TC:(ki + 1) * TC],
                                             dth[:, ki * TC:(ki + 1) * TC],
                                             AF.Ln, bias=1.0)
                    for kt in kts:
                        ki = kt - kh * KHALF
                        dts = dth[:, ki * TC:(ki + 1) * TC]
                        xcs = xc_sb[:, kt * TC:(kt + 1) * TC]
                        # u = xc * dt  (bf16 2x)
                        u = sp.tile([128, TC], BF16, tag="u", name="u", bufs=1)
                        nc.vector.tensor_tensor(u[:], xcs, dts, AL.mult)
                        # b4 = u (bcast) * bcB  (bf16 2x, one op)
                        b4 = sp.tile([128, DS * TC], BF16, tag="b4", name="b4",
                                     bufs=1)
                        uv = u[:].rearrange("q (o t) -> q o t", o=1)
                        nc.vector.tensor_tensor(
                            b4[:].rearrange("q (s t) -> q s t", s=DS),
                            uv.to_broadcast((128, DS, TC)),
                            bcB[:].rearrange("q (s t) -> q s t", s=DS), AL.mult)
                        # 16 scans: a = exp(dt*A_s) on ACT; h t-major bf16
                        h = sp.tile([128, DS * TC], F32, tag="h", name="h",
                                    bufs=1)
                        for s in range(DS):
                            a = sp.tile([128, TC], F32, tag="a", name="a",
                                        bufs=2)
                            nc.scalar.activation(
                                a[:], dts, AF.Exp,
                                scale=A_sb[:, kt * DS + s:kt * DS + s + 1])
                            init = (0.0 if tcix == 0
                                    else carry[:, kt * DS + s:kt * DS + s + 1])
                            nc.vector.tensor_tensor_scan(
                                h[:, s:s + DS * (TC - 1) + 1:DS], a[:],
                                b4[:, s * TC:(s + 1) * TC], init,
                                AL.mult, AL.add)
                        nc.vector.tensor_copy(
                            carry[:, kt * DS:(kt + 1) * DS],
                            h[:, DS * (TC - 1):DS * TC])
                        # y = sum_s C*h via contiguous mult-cumsum + diffs
                        yv = sp.tile([128, TC], F32, tag="yv", name="yv", bufs=1)
                        R = sp.tile([128, RCH * DS], F32, tag="R", name="R",
                                    bufs=1)
                        for c in range(NRC):
                            tA = c * RCH
                            nc.vector._custom_dve(
                                mscan, out=R[:],
                                in0=h[:, tA * DS:(tA + RCH) * DS],
                                in1=bcC[:, tA * DS:(tA + RCH) * DS])
                            nc.vector.tensor_copy(yv[:, tA:tA + 1],
                                                  R[:, DS - 1:DS])
                            nc.vector.tensor_tensor(
                                yv[:, tA + 1:tA + RCH], R[:, 2 * DS - 1::DS],
                                R[:, DS - 1:(RCH - 1) * DS:DS], AL.subtract)
                        nc.vector.scalar_tensor_tensor(
                            yv[:], xcs, vec["Dp"][:, kt:kt + 1], yv[:],
                            AL.mult, AL.add)
                        g = sp.tile([128, TC], MMDT, tag="g", name="g", bufs=1)
                        nc.vector.tensor_tensor(
                            g[:], yv[:], sz_sb[:, kt * TC:(kt + 1) * TC],
                            AL.mult)
                        w4 = wp.tile([128, KD * 128], MMDT, tag="w_out",
                                     name="w_out")
                        nc.sync.dma_start(
                            w4[:], dram[p + "out_wT"][kt * 128:(kt + 1) * 128, :])
                        for k in range(KD):
                            for nk in range(0, TC, MMF):
                                nn = min(MMF, TC - nk)
                                mm(y_ps[k][:, nk:nk + nn],
                                   w4[:, k * 128:(k + 1) * 128],
                                   g[:, nk:nk + nn],
                                   start=(kt == 0), stop=(kt == KI - 1))
                for k in range(KD):
                    yo = sp.tile([128, TC], F32, tag="yo", name="yo", bufs=1)
                    nc.scalar.copy(yo[:], y_ps[k][:])
                    nc.sync.dma_start(
                        y_scr[di_][k * 128:(k + 1) * 128, t0:t0 + TC], yo[:])

        # ============================================================ phase C
        with tc_.tile_pool(name="cpool", bufs=1) as cp, \
             tc_.tile_pool(name="csp", bufs=2) as sp:
            ln_g = cp.tile([128, KD], F32, tag="ln_g", name="ln_g")
            ln_b = cp.tile([128, KD], F32, tag="ln_b", name="ln_b")
            c1b = cp.tile([128, KF], F32, tag="c1b", name="c1b")
            c2b = cp.tile([128, KD], F32, tag="c2b", name="c2b")
            for nm, t in (("ln_g", ln_g), ("ln_b", ln_b), ("c1_b", c1b),
                          ("c2_b", c2b)):
                nc.sync.dma_start(t[:], dram[nm][:])
            CH = min(MMF, L)

            def ln_chunk(in_tiles, out_tiles, nk):
                """LayerNorm over D for positions [nk, nk+CH), chunk-local."""
                sps = psacc.tile([1, CH], F32, tag="mmrow", name="mmrow")
                for k in range(KD):
                    mm(sps[:], ones_sb[:, 0:1], in_tiles[k][:, nk:nk + CH],
                       start=(k == 0), stop=(k == KD - 1))
                sums = cp.tile([1, CH], MMDT, tag="ln_srow", name="ln_srow")
                nc.scalar.copy(sums[:], sps[:])
                qps = psacc.tile([1, CH], F32, tag="mmrow", name="mmrow")
                for k in range(KD):
                    sq = sp.tile([128, CH], MMDT, tag="ln_sq", name="ln_sq")
                    nc.scalar.activation(sq[:], in_tiles[k][:, nk:nk + CH],
                                         AF.Square)
                    mm(qps[:], ones_sb[:, 0:1], sq[:],
                       start=(k == 0), stop=(k == KD - 1))
                sqs = cp.tile([1, CH], MMDT, tag="ln_qrow", name="ln_qrow")
                nc.scalar.copy(sqs[:], qps[:])
                mu = cp.tile([128, CH], F32, tag="ln_mu", name="ln_mu")
                inv = cp.tile([128, CH], F32, tag="ln_inv", name="ln_inv")
                mps = psmm.tile([128, CH], F32, tag="mm", name="mm")
                mm(mps[:], ones_sb[0:1, :], sums[:], start=True, stop=True)
                nc.vector.tensor_scalar(mu[:], mps[:], 1.0 / D, None, AL.mult)
                qrep = psmm.tile([128, CH], F32, tag="mm", name="mm")
                mm(qrep[:], ones_sb[0:1, :], sqs[:], start=True, stop=True)
                ex2 = sp.tile([128, CH], F32, tag="ln_ex2", name="ln_ex2")
                nc.vector.tensor_scalar(ex2[:], qrep[:], 1.0 / D, None, AL.mult)
                var = sp.tile([128, CH], F32, tag="ln_var", name="ln_var")
                nc.vector.tensor_tensor(var[:], mu[:], mu[:], AL.mult)
                nc.vector.tensor_tensor(var[:], ex2[:], var[:], AL.subtract)
                sd = sp.tile([128, CH], F32, tag="ln_sd", name="ln_sd")
                nc.scalar.activation(sd[:], var[:], AF.Sqrt, bias=eps_sb[:])
                nc.vector.reciprocal(inv[:], sd[:])
                for k in range(KD):
                    xm = sp.tile([128, CH], F32, tag="ln_xm", name="ln_xm")
                    nc.vector.tensor_tensor(xm[:], in_tiles[k][:, nk:nk + CH],
                                            mu[:], AL.subtract)
                    nc.vector.tensor_tensor(xm[:], xm[:], inv[:], AL.mult)
                    nc.vector.tensor_scalar(out_tiles[k][:, nk:nk + CH], xm[:],
                                            ln_g[:, k:k + 1], ln_b[:, k:k + 1],
                                            AL.mult, AL.add)

            y3p = [cp.tile([128, L], MMDT, tag=f"y3p{k}", name=f"y3p{k}")
                   for k in range(KD)]
            y3 = [cp.tile([128, L], MMDT, tag=f"y3_{k}", name=f"y3_{k}")
                  for k in range(KD)]
            outs = [cp.tile([128, L], MMDT, tag=f"o_{k}", name=f"o_{k}")
                    for k in range(KD)]
            ypre = y3p
            NFH = min(8, KF)
            for nk in range(0, L, CH):
                for k in range(KD):
                    xt = sp.tile([128, CH], MMDT, tag="c_x", name="c_x")
                    y1t = sp.tile([128, CH], F32, tag="c_y1", name="c_y1")
                    y2t = sp.tile([128, CH], F32, tag="c_y2", name="c_y2")
                    nc.sync.dma_start(
                        xt[:], dram["xT"][k * 128:(k + 1) * 128, nk:nk + CH])
                    nc.sync.dma_start(
                        y1t[:], y_scr[0][k * 128:(k + 1) * 128, nk:nk + CH])
                    nc.sync.dma_start(
                        y2t[:], y_scr[1][k * 128:(k + 1) * 128,
                                         L - nk - CH:L - nk])
                    nc.vector.tensor_tensor(y3p[k][:, nk:nk + CH], xt[:],
                                            y1t[:], AL.add)
                    nc.vector.tensor_tensor(y3p[k][:, nk:nk + CH],
                                            y3p[k][:, nk:nk + CH],
                                            y2t[:, ::-1], AL.add)
                ln_chunk(y3p, y3, nk)
                yacc = [psacc.tile([128, CH], F32, tag=f"acc{k}", name=f"acc{k}")
                        for k in range(KD)]
                for fh in range(KF // NFH):
                    hbuf = []
                    for f2 in range(NFH):
                        f = fh * NFH + f2
                        hps = psmm.tile([128, CH], F32, tag="mm", name="mm")
                        wc1 = wp.tile([128, KD * 128], MMDT, tag="w_in",
                                      name="w_c1")
                        nc.sync.dma_start(
                            wc1[:].rearrange("q (k e) -> q k e", k=KD),
                            dram["c1_wT"].rearrange("(k q) e -> q k e", q=128)
                            [:, :, f * 128:(f + 1) * 128])
                        for k in range(KD):
                            mm(hps[:], wc1[:, k * 128:(k + 1) * 128],
                               y3[k][:, nk:nk + CH],
                               start=(k == 0), stop=(k == KD - 1))
                        hb = sp.tile([128, CH], MMDT, tag=f"hb{f2}",
                                     name=f"hb{f2}", bufs=1)
                        nc.scalar.activation(hb[:], hps[:], AF.Relu,
                                             bias=c1b[:, f:f + 1])
                        hbuf.append(hb)
                    for f2 in range(NFH):
                        f = fh * NFH + f2
                        wc2 = wp.tile([128, KD * 128], MMDT, tag="w_out",
                                      name="w_c2")
                        nc.sync.dma_start(wc2[:],
                                          dram["c2_wT"][f * 128:(f + 1) * 128, :])
                        for k in range(KD):
                            mm(yacc[k][:], wc2[:, k * 128:(k + 1) * 128],
                               hbuf[f2][:],
                               start=(f == 0), stop=(f == KF - 1))
                for k in range(KD):
                    nc.vector.scalar_tensor_tensor(
                        ypre[k][:, nk:nk + CH], yacc[k][:], c2b[:, k:k + 1],
                        y3[k][:, nk:nk + CH], AL.add, AL.add)
                ln_chunk(ypre, outs, nk)
                for k in range(KD):
                    nc.sync.dma_start(
                        outT[k * 128:(k + 1) * 128, nk:nk + CH],
                        outs[k][:, nk:nk + CH].bitcast(F32))


# ------------------------------------------------------------------ host side
_PROG_CACHE = {}


def _get_prog():
    if "full" not in _PROG_CACHE:
        _PROG_CACHE["full"] = build_program(DIMS)
    return _PROG_CACHE["full"]


def host_prep(inputs, dm: Dims = DIMS):
    f = np.float32
    x = np.asarray(inputs["x"], dtype=f)
    KI, KD, KF = dm.KI, dm.KD, dm.KF

    def vt(v, n):
        return np.ascontiguousarray(np.asarray(v, f).reshape(n, 128).T)

    c = {}
    sel = np.zeros((dm.DTR, 2 * dm.DS * 128), f)
    for s in range(2 * dm.DS):
        sel[s, s * 128:(s + 1) * 128] = 1.0
    c["sel"] = sel
    c["ones"] = np.ones((128, 128), f)
    for p in ("m1_", "m2_"):
        c[p + "in_wT"] = np.ascontiguousarray(np.asarray(inputs[p + "in_w"], f).T)
        c[p + "xproj_wT"] = np.ascontiguousarray(
            np.asarray(inputs[p + "xproj_w"], f).T)
        c[p + "dt_wT"] = np.ascontiguousarray(np.asarray(inputs[p + "dt_w"], f).T)
        c[p + "out_wT"] = np.ascontiguousarray(
            np.asarray(inputs[p + "out_w"], f).T)
        c[p + "A"] = np.ascontiguousarray(-np.exp(np.asarray(inputs[p + "A_log"], f)))
        c[p + "dt_b"] = vt(inputs[p + "dt_b"], KI)
        cw = np.asarray(inputs[p + "conv_w"], f)
        c[p + "cw0"] = vt(cw[:, 0], KI)
        c[p + "cw1"] = vt(cw[:, 1], KI)
        c[p + "cb"] = vt(inputs[p + "conv_b"], KI)
        c[p + "Dp"] = vt(np.asarray(inputs[p + "Dp"], f), KI)
    c["ln_g"] = vt(inputs["ln_g"], KD)
    c["ln_b"] = vt(inputs["ln_b"], KD)
    c["c1_wT"] = np.ascontiguousarray(np.asarray(inputs["c1_w"], f).T)
    c["c1_b"] = vt(inputs["c1_b"], KF)
    c["c2_wT"] = np.ascontiguousarray(np.asarray(inputs["c2_w"], f).T)
    c["c2_b"] = vt(inputs["c2_b"], KD)

    in_maps = []
    for b in range(x.shape[0]):
        m = dict(c)
        m["xT"] = np.ascontiguousarray(x[b].T)
        m["xTr"] = np.ascontiguousarray(x[b][::-1].T)
        in_maps.append(m)
    return in_maps


def kernel(**inputs):
    nc = _get_prog()
    in_maps = host_prep(inputs)
    res = bass_utils.run_bass_kernel_spmd(nc, in_maps, core_ids=list(range(NCORES)))
    return np.stack([np.ascontiguousarray(o["outT"].T) for o in res.results], axis=0)


# revision 38
# speedup vs baseline: 1.0074x; 1.0031x over previous
"""Bidirectional Mamba block — Trainium2 Bass/Tile kernel, 8-core data-parallel.

Sharding: batch B=8 -> one sample per NeuronCore, zero collectives.

v2 design (measured-rate driven):
- h stored t-major (bf16) via strided tensor_tensor_scan writes (strides are
  free on the scan's 2-cyc/elem cadence), so the C*h mult-cumsum custom op
  streams contiguously at 1 cyc/elem instead of 2.2.
- b4 = u*B and u = xc*dt run in bf16 2x_1p packed mode (0.53 ns/elem).
- P1 conv + silu moved to ACT (native Silu table); DVE does one add per kt.
- softplus = Exp then Ln on ACT (same table set as the scan's exp).
- xc / silu(z) / dt stay SBUF-resident in bf16 (no DRAM roundtrip).
- All DMA on hardware DGE queues (sync/scalar) — no gpsimd SWDGE (its
  descriptor generation contends with DVE 2-port ops for the shared port).
"""

import numpy as np

import concourse.bass as bass
import concourse.bacc as bacc
import concourse.mybir as mybir
from concourse import tile
from concourse import bass_utils

AL = mybir.AluOpType
AF = mybir.ActivationFunctionType
F32 = mybir.dt.float32
F32R = mybir.dt.float32r
BF16 = mybir.dt.bfloat16

NCORES = 8
MMF = 512
MMDT = F32R


class Dims:
    def __init__(self, L=1024, D=512, DI=2048, DS=16, DTR=32, DFF=2048, TC=512):
        self.L, self.D, self.DI, self.DS, self.DTR, self.DFF = L, D, DI, DS, DTR, DFF
        self.TC = TC
        self.NTC = L // TC
        self.KD = D // 128
        self.KI = DI // 128
        self.KF = DFF // 128
        self.RCH = 128 if TC % 128 == 0 else 64   # t-columns per mscan chunk
        assert TC % self.RCH == 0 and L % TC == 0 and DS == 16 and DTR == 32


DIMS = Dims()

# --------------------------------------------------------------- custom DVE op
_MSCAN = None


def _get_mscan():
    """out[k] = running_sum(in0[k] * in1[k]) along the flattened free stream."""
    global _MSCAN
    if _MSCAN is not None:
        return _MSCAN
    from concourse.dve_spec import Spec, Src0, Src1, scan as dve_scan, lower, AluOp
    from concourse.dve_ops import DveOp, OPS, CUSTOM_DVE_SPECS
    import concourse.dve_ops as dve_ops_mod
    from concourse.dve_uop import DveOpSpec

    name = "MAMBA_MULT_CUMSUM"
    for op in OPS:
        if op.name == name:
            _MSCAN = op
            return op

    def _ref(in0, in1, s0, s1, imm2):
        p = in0.shape[0]
        prod = np.asarray(in0, np.float32) * np.asarray(in1, np.float32)
        return np.cumsum(prod.reshape(p, -1), axis=1,
                         dtype=np.float32).reshape(in0.shape)

    spec = Spec(body=dve_scan(AluOp.ADD, Src0 * Src1), reference=_ref)
    row = max(dve_ops_mod._SUB_OPCODE_FOR_NAME.values()) + 1
    assert row < 0x20
    dve_ops_mod._SUB_OPCODE_FOR_NAME[name] = row
    shas = {}
    for ver in ("v3", "v4"):
        shas[ver] = DveOpSpec(name=name, opcode=row, uops=lower(spec, ver=ver),
                              rd1_en=True).sha(ver)
    final = DveOp(name, spec, subdim=False, uops_sha=shas)
    OPS.append(final)
    CUSTOM_DVE_SPECS[name] = spec
    _MSCAN = final
    return final


# -------------------------------------------------------------------- builder
def build_program(dm: Dims = DIMS):
    mscan = _get_mscan()
    nc = bacc.Bacc("TRN2", target_bir_lowering=False, debug=False)

    L, D, DI, DS, DTR = dm.L, dm.D, dm.DI, dm.DS, dm.DTR
    dram = {}

    def din(name, shape, dt=F32):
        dram[name] = nc.dram_tensor(name, list(shape), dt,
                                    kind="ExternalInput").ap()

    din("xT", (D, L), MMDT); din("xTr", (D, L), MMDT)
    din("ones", (128, 128), MMDT)
    din("sel", (DTR, 2 * DS * 128), MMDT)
    for p in ("m1_", "m2_"):
        din(p + "in_wT", (D, 2 * DI), MMDT)
        din(p + "xproj_wT", (DI, DTR + 2 * DS), MMDT)
        din(p + "dt_wT", (DTR, DI), MMDT)
        din(p + "out_wT", (DI, D), MMDT)
        din(p + "A", (DI, DS))                      # -exp(A_log)
        din(p + "dt_b", (128, dm.KI))
        din(p + "cw0", (128, dm.KI))
        din(p + "cw1", (128, dm.KI))
        din(p + "cb", (128, dm.KI))
        din(p + "Dp", (128, dm.KI))
    din("ln_g", (128, dm.KD)); din("ln_b", (128, dm.KD))
    din("c1_wT", (D, dm.DFF), MMDT); din("c1_b", (128, dm.KF))
    din("c2_wT", (dm.DFF, D), MMDT); din("c2_b", (128, dm.KD))
    outT = nc.dram_tensor("outT", [D, L], F32, kind="ExternalOutput").ap()

    with tile.TileContext(nc) as tc_:
        _emit(nc, tc_, dram, outT, dm, mscan)
    nc.compile()
    return nc


def _emit(nc, tc_, dram, outT, dm, mscan):
    from contextlib import ExitStack
    L, D, DI, DS, DTR, DFF, TC, NTC = (dm.L, dm.D, dm.DI, dm.DS, dm.DTR,
                                       dm.DFF, dm.TC, dm.NTC)
    KD, KI, KF, RCH = dm.KD, dm.KI, dm.KF, dm.RCH
    NRC = TC // RCH
    KHALF = max(1, min(8, KI // 2))
    mm = nc.tensor.matmul

    with ExitStack() as ctx:
        pers = ctx.enter_context(tc_.tile_pool(name="pers", bufs=1))
        wp = ctx.enter_context(tc_.tile_pool(name="wp", bufs=2))
        psmm = ctx.enter_context(tc_.tile_pool(name="psmm", bufs=2, space="PSUM"))
        psacc = ctx.enter_context(tc_.tile_pool(name="psacc", bufs=1, space="PSUM"))
        dpool = ctx.enter_context(tc_.tile_pool(name="dpool", bufs=1, space="DRAM"))

        ones_sb = pers.tile([128, 128], MMDT, tag="ones", name="ones")
        nc.sync.dma_start(ones_sb[:], dram["ones"][:])
        eps_sb = pers.tile([128, 1], F32, tag="eps", name="eps")
        nc.vector.memset(eps_sb[:], 1e-5)

        y_scr = [dpool.tile([D, L], F32, tag=f"y_scr{i}", name=f"y_scr{i}")
                 for i in range(2)]

        # ====================================================== SSM directions
        for di_ in range(2):
          with tc_.tile_pool(name=f"dirp{di_}", bufs=1) as dirp, \
               tc_.tile_pool(name=f"dsp{di_}", bufs=2) as sp:
            p = ("m1_", "m2_")[di_]
            xnm = ("xT", "xTr")[di_]
            A_sb = dirp.tile([128, KI * DS], F32, tag="A", name="A")
            nc.sync.dma_start(
                A_sb[:].rearrange("q (k s) -> q k s", k=KI),
                dram[p + "A"].rearrange("(k q) s -> q k s", q=128))
            vec = {}
            for nm in ("dt_b", "cw0", "cw1", "cb", "Dp"):
                vec[nm] = dirp.tile([128, KI], F32, tag=nm, name=nm)
                nc.sync.dma_start(vec[nm][:], dram[p + nm][:])
            xpw_sb = dirp.tile([128, KI * (DTR + 2 * DS)], MMDT, tag="xpw",
                               name="xpw")
            nc.sync.dma_start(
                xpw_sb[:].rearrange("q (k c) -> q k c", k=KI),
                dram[p + "xproj_wT"].rearrange("(k q) c -> q k c", q=128))

            sel_sb = dirp.tile([64, 2 * DS * 128], MMDT, tag="sel", name="sel")
            nc.sync.dma_start(sel_sb[32:64, :], dram["sel"][:])
            dtw_sb = dirp.tile([DTR, DI], MMDT, tag="dtw", name="dtw")
            nc.sync.dma_start(dtw_sb[:], dram[p + "dt_wT"][:])
            carry = dirp.tile([128, KI * DS], BF16, tag="carry", name="carry")
            carryB = dirp.tile([128, KI], F32, tag="carryB", name="carryB")
            nc.vector.memset(carryB[:], 0.0)
            bcB = dirp.tile([128, DS * TC], BF16, tag="bcB", name="bcB")
            bcC = dirp.tile([128, DS * TC], BF16, tag="bcC", name="bcC")
            # bf16 per-direction activations, SBUF-resident
            xc_sb = dirp.tile([128, KI * TC], BF16, tag="xc_sb", name="xc_sb")
            sz_sb = dirp.tile([128, KI * TC], BF16, tag="sz_sb", name="sz_sb")
            dt_h = [dirp.tile([128, KHALF * TC], BF16, tag=f"dt_h{i}",
                              name=f"dt_h{i}") for i in range(2)]
            dbc_sb = dirp.tile([64, TC], MMDT, tag="dbc", name="dbc")

            for tcix in range(NTC):
                t0 = tcix * TC
                xtc = [sp.tile([128, TC], MMDT, tag=f"xtc{k}", name=f"xtc{k}",
                               bufs=1) for k in range(KD)]
                for k in range(KD):
                    nc.sync.dma_start(xtc[k][:],
                                      dram[xnm][k * 128:(k + 1) * 128, t0:t0 + TC])

                # ---- P1: x-part conv + silu; z-part silu; dbc ---------------
                dbc_ps = psacc.tile([64, TC], F32, tag="acc_dbc", name="acc_dbc")
                for kt in range(KI):
                    ps = psmm.tile([128, TC], F32, tag="mm", name="mm")
                    w4 = wp.tile([128, KD * 128], MMDT, tag="w_in", name="w_in")
                    nc.sync.dma_start(
                        w4[:].rearrange("q (k e) -> q k e", k=KD),
                        dram[p + "in_wT"].rearrange("(k q) e -> q k e", q=128)
                        [:, :, kt * 128:(kt + 1) * 128])
                    for nk in range(0, TC, MMF):
                        nn = min(MMF, TC - nk)
                        for k in range(KD):
                            mm(ps[:, nk:nk + nn], w4[:, k * 128:(k + 1) * 128],
                               xtc[k][:, nk:nk + nn],
                               start=(k == 0), stop=(k == KD - 1))
                    # s1 = ps*cw1 + cb   (ACT, PSUM->SBUF)
                    s1 = sp.tile([128, TC], F32, tag="cv1", name="cv1", bufs=1)
                    nc.scalar.activation(s1[:], ps[:], AF.Identity,
                                         bias=vec["cb"][:, kt:kt + 1],
                                         scale=vec["cw1"][:, kt:kt + 1])
                    # p0 = ps*cw0        (ACT, PSUM->SBUF)
                    p0 = sp.tile([128, TC], F32, tag="cv0", name="cv0", bufs=1)
                    nc.scalar.activation(p0[:], ps[:], AF.Copy,
                                         scale=vec["cw0"][:, kt:kt + 1])
                    # v = shift(p0) + s1 (DVE)
                    v = sp.tile([128, TC], F32, tag="cv2", name="cv2", bufs=2)
                    nc.vector.tensor_tensor(v[:, 1:TC], p0[:, 0:TC - 1],
                                            s1[:, 1:TC], AL.add)
                    nc.vector.tensor_tensor(v[:, 0:1], carryB[:, kt:kt + 1],
                                            s1[:, 0:1], AL.add)
                    nc.vector.tensor_copy(carryB[:, kt:kt + 1], p0[:, TC - 1:TC])
                    # xc = silu(v): f32r for the dbc matmul; bf16 via DVE cast
                    xck = sp.tile([128, TC], MMDT, tag="xck", name="xck", bufs=2)
                    nc.scalar.activation(xck[:], v[:], AF.Silu)
                    nc.vector.tensor_copy(xc_sb[:, kt * TC:(kt + 1) * TC],
                                          xck[:].bitcast(F32))
                    for nk in range(0, TC, MMF):
                        nn = min(MMF, TC - nk)
                        mm(dbc_ps[:, nk:nk + nn],
                           xpw_sb[:, kt * 64:(kt + 1) * 64],
                           xck[:, nk:nk + nn],
                           start=(kt == 0), stop=(kt == KI - 1))
                nc.scalar.copy(dbc_sb[:], dbc_ps[:])

                # ---- B3: broadcast B (s-major bf16) / C (t-major bf16) ------
                for s in range(2 * DS):
                    bps = psmm.tile([128, TC], F32, tag="mm", name="mm")
                    for nk in range(0, TC, MMF):
                        nn = min(MMF, TC - nk)
                        mm(bps[:, nk:nk + nn],
                           sel_sb[32:64, s * 128:(s + 1) * 128],
                           dbc_sb[DTR:DTR + 2 * DS, nk:nk + nn],
                           start=True, stop=True)
                    if s < DS:
                        nc.scalar.activation(bcB[:, s * TC:(s + 1) * TC],
                                             bps[:], AF.Copy)
                    else:
                        si = s - DS
                        nc.scalar.activation(
                            bcC[:].rearrange("q (t s) -> q s t", s=DS)[:, si, :],
                            bps[:], AF.Copy)

                for kt in range(KI):           # z-part: silu only
                    zps = psmm.tile([128, TC], F32, tag="mm", name="mm")
                    w4 = wp.tile([128, KD * 128], MMDT, tag="w_in", name="w_in")
                    nc.sync.dma_start(
                        w4[:].rearrange("q (k e) -> q k e", k=KD),
                        dram[p + "in_wT"].rearrange("(k q) e -> q k e", q=128)
                        [:, :, DI + kt * 128:DI + (kt + 1) * 128])
                    for nk in range(0, TC, MMF):
                        nn = min(MMF, TC - nk)
                        for k in range(KD):
                            mm(zps[:, nk:nk + nn], w4[:, k * 128:(k + 1) * 128],
                               xtc[k][:, nk:nk + nn],
                               start=(k == 0), stop=(k == KD - 1))
                    nc.scalar.activation(sz_sb[:, kt * TC:(kt + 1) * TC],
                                         zps[:], AF.Silu)
                # ---- P2 per kt-half ----------------------------------------
                y_ps = [psacc.tile([128, TC], F32, tag=f"acc{k}", name=f"acc{k}")
                        for k in range(KD)]
                nhalves = (KI + KHALF - 1) // KHALF
                for kh in range(nhalves):
                    kts = range(kh * KHALF, min(KI, (kh + 1) * KHALF))
                    dth = dt_h[kh % 2]
                    for kt in kts:          # dt = softplus: Exp batch ...
                        ki = kt - kh * KHALF
                        dps = psmm.tile([128, TC], F32, tag="mm", name="mm")
                        for nk in range(0, TC, MMF):
                            nn = min(MMF, TC - nk)
                            mm(dps[:, nk:nk + nn],
                               dtw_sb[:, kt * 128:(kt + 1) * 128],
                               dbc_sb[0:DTR, nk:nk + nn],
                               start=True, stop=True)
                        nc.scalar.activation(dth[:, ki * TC:(ki + 1) * TC],
                                             dps[:], AF.Exp,
                                             bias=vec["dt_b"][:, kt:kt + 1])
                    for kt in kts:          # ... then Ln batch, in place
                        ki = kt - kh * KHALF
                        nc.scalar.activation(dth[:, ki * TC:(ki + 1) * TC],
                                             dth[:, ki * TC:(ki + 1) * TC],
                                             AF.Ln, bias=1.0)
                    for kt in kts:
                        ki = kt - kh * KHALF
                        dts = dth[:, ki * TC:(ki + 1) * TC]
                        xcs = xc_sb[:, kt * TC:(kt + 1) * TC]
                        # u = xc * dt  (bf16 2x)
                        u = sp.tile([128, TC], BF16, tag="u", name="u", bufs=1)
                        nc.vector.tensor_tensor(u[:], xcs, dts, AL.mult)
                        # b4 = u (bcast) * bcB  (bf16 2x, one op)
                        b4 = sp.tile([128, DS * TC], BF16, tag="b4", name="b4",
                                     bufs=1)
                        uv = u[:].rearrange("q (o t) -> q o t", o=1)
                        nc.vector.tensor_tensor(
                            b4[:].rearrange("q (s t) -> q s t", s=DS),
                            uv.to_broadcast((128, DS, TC)),
                            bcB[:].rearrange("q (s t) -> q s t", s=DS), AL.mult)
                        # 16 scans: a = exp(dt*A_s) on ACT; h t-major bf16
                        h = sp.tile([128, DS * TC], F32, tag="h", name="h",
                                    bufs=1)
                        for s in range(DS):
                            a = sp.tile([128, TC], F32, tag="a", name="a",
                                        bufs=2)
                            nc.scalar.activation(
                                a[:], dts, AF.Exp,
                                scale=A_sb[:, kt * DS + s:kt * DS + s + 1])
                            init = (0.0 if tcix == 0
                                    else carry[:, kt * DS + s:kt * DS + s + 1])
                            nc.vector.tensor_tensor_scan(
                                h[:, s:s + DS * (TC - 1) + 1:DS], a[:],
                                b4[:, s * TC:(s + 1) * TC], init,
                                AL.mult, AL.add)
                        nc.vector.tensor_copy(
                            carry[:, kt * DS:(kt + 1) * DS],
                            h[:, DS * (TC - 1):DS * TC])
                        # y = sum_s C*h via contiguous mult-cumsum + diffs
                        yv = sp.tile([128, TC], F32, tag="yv", name="yv", bufs=1)
                        R = sp.tile([128, RCH * DS], F32, tag="R", name="R",
                                    bufs=1)
                        for c in range(NRC):
                            tA = c * RCH
                            nc.vector._custom_dve(
                                mscan, out=R[:],
                                in0=h[:, tA * DS:(tA + RCH) * DS],
                                in1=bcC[:, tA * DS:(tA + RCH) * DS])
                            nc.vector.tensor_copy(yv[:, tA:tA + 1],
                                                  R[:, DS - 1:DS])
                            nc.vector.tensor_tensor(
                                yv[:, tA + 1:tA + RCH], R[:, 2 * DS - 1::DS],
                                R[:, DS - 1:(RCH - 1) * DS:DS], AL.subtract)
                        nc.vector.scalar_tensor_tensor(
                            yv[:], xcs, vec["Dp"][:, kt:kt + 1], yv[:],
                            AL.mult, AL.add)
                        g = sp.tile([128, TC], MMDT, tag="g", name="g", bufs=1)
                        nc.vector.tensor_tensor(
                            g[:], yv[:], sz_sb[:, kt * TC:(kt + 1) * TC],
                            AL.mult)
                        w4 = wp.tile([128, KD * 128], MMDT, tag="w_out",
                                     name="w_out")
                        nc.sync.dma_start(
                            w4[:], dram[p + "out_wT"][kt * 128:(kt + 1) * 128, :])
                        for k in range(KD):
                            for nk in range(0, TC, MMF):
                                nn = min(MMF, TC - nk)
                                mm(y_ps[k][:, nk:nk + nn],
                                   w4[:, k * 128:(k + 1) * 128],
                                   g[:, nk:nk + nn],
                                   start=(kt == 0), stop=(kt == KI - 1))
                for k in range(KD):
                    yo = sp.tile([128, TC], F32, tag="yo", name="yo", bufs=1)
                    nc.scalar.copy(yo[:], y_ps[k][:])
                    nc.sync.dma_start(
                        y_scr[di_][k * 128:(k + 1) * 128, t0:t0 + TC], yo[:])

        # ============================================================ phase C
        with tc_.tile_pool(name="cpool", bufs=1) as cp, \
             tc_.tile_pool(name="csp", bufs=2) as sp:
            ln_g = cp.tile([128, KD], F32, tag="ln_g", name="ln_g")
            ln_b = cp.tile([128, KD], F32, tag="ln_b", name="ln_b")
            c1b = cp.tile([128, KF], F32, tag="c1b", name="c1b")
            c2b = cp.tile([128, KD], F32, tag="c2b", name="c2b")
            for nm, t in (("ln_g", ln_g), ("ln_b", ln_b), ("c1_b", c1b),
                          ("c2_b", c2b)):
                nc.sync.dma_start(t[:], dram[nm][:])
            CH = min(MMF, L)

            def ln_chunk(in_tiles, out_tiles, nk):
                """LayerNorm over D for positions [nk, nk+CH), chunk-local."""
                sps = psacc.tile([1, CH], F32, tag="mmrow", name="mmrow")
                for k in range(KD):
                    mm(sps[:], ones_sb[:, 0:1], in_tiles[k][:, nk:nk + CH],
                       start=(k == 0), stop=(k == KD - 1))
                sums = cp.tile([1, CH], MMDT, tag="ln_srow", name="ln_srow")
                nc.scalar.copy(sums[:], sps[:])
                qps = psacc.tile([1, CH], F32, tag="mmrow", name="mmrow")
                for k in range(KD):
                    sq = sp.tile([128, CH], MMDT, tag="ln_sq", name="ln_sq")
                    nc.scalar.activation(sq[:], in_tiles[k][:, nk:nk + CH],
                                         AF.Square)
                    mm(qps[:], ones_sb[:, 0:1], sq[:],
                       start=(k == 0), stop=(k == KD - 1))
                sqs = cp.tile([1, CH], MMDT, tag="ln_qrow", name="ln_qrow")
                nc.scalar.copy(sqs[:], qps[:])
                mu = cp.tile([128, CH], F32, tag="ln_mu", name="ln_mu")
                inv = cp.tile([128, CH], F32, tag="ln_inv", name="ln_inv")
                mps = psmm.tile([128, CH], F32, tag="mm", name="mm")
                mm(mps[:], ones_sb[0:1, :], sums[:], start=True, stop=True)
                nc.vector.tensor_scalar(mu[:], mps[:], 1.0 / D, None, AL.mult)
                qrep = psmm.tile([128, CH], F32, tag="mm", name="mm")
                mm(qrep[:], ones_sb[0:1, :], sqs[:], start=True, stop=True)
                ex2 = sp.tile([128, CH], F32, tag="ln_ex2", name="ln_ex2")
                nc.vector.tensor_scalar(ex2[:], qrep[:], 1.0 / D, None, AL.mult)
                var = sp.tile([128, CH], F32, tag="ln_var", name="ln_var")
                nc.vector.tensor_tensor(var[:], mu[:], mu[:], AL.mult)
                nc.vector.tensor_tensor(var[:], ex2[:], var[:], AL.subtract)
                sd = sp.tile([128, CH], F32, tag="ln_sd", name="ln_sd")
                nc.scalar.activation(sd[:], var[:], AF.Sqrt, bias=eps_sb[:])
                nc.vector.reciprocal(inv[:], sd[:])
                for k in range(KD):
                    xm = sp.tile([128, CH], F32, tag="ln_xm", name="ln_xm")
                    nc.vector.tensor_tensor(xm[:], in_tiles[k][:, nk:nk + CH],
                                            mu[:], AL.subtract)
                    nc.vector.tensor_tensor(xm[:], xm[:], inv[:], AL.mult)
                    nc.vector.tensor_scalar(out_tiles[k][:, nk:nk + CH], xm[:],
                                            ln_g[:, k:k + 1], ln_b[:, k:k + 1],
                                            AL.mult, AL.add)

            y3p = [cp.tile([128, L], MMDT, tag=f"y3p{k}", name=f"y3p{k}")
                   for k in range(KD)]
            y3 = [cp.tile([128, L], MMDT, tag=f"y3_{k}", name=f"y3_{k}")
                  for k in range(KD)]
            outs = [cp.tile([128, L], MMDT, tag=f"o_{k}", name=f"o_{k}")
                    for k in range(KD)]
            ypre = y3p
            NFH = min(8, KF)
            for nk in range(0, L, CH):
                for k in range(KD):
                    xt = sp.tile([128, CH], MMDT, tag="c_x", name="c_x")
                    y1t = sp.tile([128, CH], F32, tag="c_y1", name="c_y1")
                    y2t = sp.tile([128, CH], F32, tag="c_y2", name="c_y2")
                    nc.sync.dma_start(
                        xt[:], dram["xT"][k * 128:(k + 1) * 128, nk:nk + CH])
                    nc.sync.dma_start(
                        y1t[:], y_scr[0][k * 128:(k + 1) * 128, nk:nk + CH])
                    nc.sync.dma_start(
                        y2t[:], y_scr[1][k * 128:(k + 1) * 128,
                                         L - nk - CH:L - nk])
                    nc.vector.tensor_tensor(y3p[k][:, nk:nk + CH], xt[:],
                                            y1t[:], AL.add)
                    nc.vector.tensor_tensor(y3p[k][:, nk:nk + CH],
                                            y3p[k][:, nk:nk + CH],
                                            y2t[:, ::-1], AL.add)
                ln_chunk(y3p, y3, nk)
            for nk in range(0, L, CH):
                yacc = [psacc.tile([128, CH], F32, tag=f"acc{k}", name=f"acc{k}")
                        for k in range(KD)]
                for fh in range(KF // NFH):
                    hbuf = []
                    for f2 in range(NFH):
                        f = fh * NFH + f2
                        hps = psmm.tile([128, CH], F32, tag="mm", name="mm")
                        wc1 = wp.tile([128, KD * 128], MMDT, tag="w_in",
                                      name="w_c1")
                        nc.sync.dma_start(
                            wc1[:].rearrange("q (k e) -> q k e", k=KD),
                            dram["c1_wT"].rearrange("(k q) e -> q k e", q=128)
                            [:, :, f * 128:(f + 1) * 128])
                        for k in range(KD):
                            mm(hps[:], wc1[:, k * 128:(k + 1) * 128],
                               y3[k][:, nk:nk + CH],
                               start=(k == 0), stop=(k == KD - 1))
                        hb = sp.tile([128, CH], MMDT, tag=f"hb{f2}",
                                     name=f"hb{f2}", bufs=1)
                        nc.scalar.activation(hb[:], hps[:], AF.Relu,
                                             bias=c1b[:, f:f + 1])
                        hbuf.append(hb)
                    for f2 in range(NFH):
                        f = fh * NFH + f2
                        wc2 = wp.tile([128, KD * 128], MMDT, tag="w_out",
                                      name="w_c2")
                        nc.sync.dma_start(wc2[:],
                                          dram["c2_wT"][f * 128:(f + 1) * 128, :])
                        for k in range(KD):
                            mm(yacc[k][:], wc2[:, k * 128:(k + 1) * 128],
                               hbuf[f2][:],
                               start=(f == 0), stop=(f == KF - 1))
                for k in range(KD):
                    nc.vector.scalar_tensor_tensor(
                        ypre[k][:, nk:nk + CH], yacc[k][:], c2b[:, k:k + 1],
                        y3[k][:, nk:nk + CH], AL.add, AL.add)
                ln_chunk(ypre, outs, nk)
                for k in range(KD):
                    nc.sync.dma_start(
                        outT[k * 128:(k + 1) * 128, nk:nk + CH],
                        outs[k][:, nk:nk + CH].bitcast(F32))


# ------------------------------------------------------------------ host side
_PROG_CACHE = {}


def _get_prog():
    if "full" not in _PROG_CACHE:
        _PROG_CACHE["full"] = build_program(DIMS)
    return _PROG_CACHE["full"]


def host_prep(inputs, dm: Dims = DIMS):
    f = np.float32
    x = np.asarray(inputs["x"], dtype=f)
    KI, KD, KF = dm.KI, dm.KD, dm.KF

    def vt(v, n):
        return np.ascontiguousarray(np.asarray(v, f).reshape(n, 128).T)

    c = {}
    sel = np.zeros((dm.DTR, 2 * dm.DS * 128), f)
    for s in range(2 * dm.DS):
        sel[s, s * 128:(s + 1) * 128] = 1.0
    c["sel"] = sel
    c["ones"] = np.ones((128, 128), f)
    for p in ("m1_", "m2_"):
        c[p + "in_wT"] = np.ascontiguousarray(np.asarray(inputs[p + "in_w"], f).T)
        c[p + "xproj_wT"] = np.ascontiguousarray(
            np.asarray(inputs[p + "xproj_w"], f).T)
        c[p + "dt_wT"] = np.ascontiguousarray(np.asarray(inputs[p + "dt_w"], f).T)
        c[p + "out_wT"] = np.ascontiguousarray(
            np.asarray(inputs[p + "out_w"], f).T)
        c[p + "A"] = np.ascontiguousarray(-np.exp(np.asarray(inputs[p + "A_log"], f)))
        c[p + "dt_b"] = vt(inputs[p + "dt_b"], KI)
        cw = np.asarray(inputs[p + "conv_w"], f)
        c[p + "cw0"] = vt(cw[:, 0], KI)
        c[p + "cw1"] = vt(cw[:, 1], KI)
        c[p + "cb"] = vt(inputs[p + "conv_b"], KI)
        c[p + "Dp"] = vt(np.asarray(inputs[p + "Dp"], f), KI)
    c["ln_g"] = vt(inputs["ln_g"], KD)
    c["ln_b"] = vt(inputs["ln_b"], KD)
    c["c1_wT"] = np.ascontiguousarray(np.asarray(inputs["c1_w"], f).T)
    c["c1_b"] = vt(inputs["c1_b"], KF)
    c["c2_wT"] = np.ascontiguousarray(np.asarray(inputs["c2_w"], f).T)
    c["c2_b"] = vt(inputs["c2_b"], KD)

    in_maps = []
    for b in range(x.shape[0]):
        m = dict(c)
        m["xT"] = np.ascontiguousarray(x[b].T)
        m["xTr"] = np.ascontiguousarray(x[b][::-1].T)
        in_maps.append(m)
    return in_maps


def kernel(**inputs):
    nc = _get_prog()
    in_maps = host_prep(inputs)
    res = bass_utils.run_bass_kernel_spmd(nc, in_maps, core_ids=list(range(NCORES)))
    return np.stack([np.ascontiguousarray(o["outT"].T) for o in res.results], axis=0)
